# revision 12
# baseline (speedup 1.0000x reference)
"""Trainium2 Bass kernel for ClassicalSelfAttention.

  out = softmax((x @ Wq) @ (x @ Wk)^T / sqrt(D)) @ x      x: [8192, 1024] f32

Algebraic restructuring 1 (weight folding): scores = x (Wq Wk^T) x^T; the
weight matrices fold offline into Z = Wq Wk^T.  Each core projects only
its own row-shard (q~ = x_own @ 8Z) and computes its scores row-block
directly against x^T streamed in fp8.  No K projection, no K^T AllGather.

Algebraic restructuring 2 (LINEAR SPLIT): the logits are small
(l ~ N(0, 0.41^2)), so exp(l) = 1 + l + r(l) with the curvature
remainder r = e^l - 1 - l tiny (rms ~0.12 vs ~1.18 for e^l).  The
attention numerator splits exactly:

   P V = mu * colsum(V)  +  q~ (X^T X) / 256  +  R V

The linear term collapses through the D x D Gram matrix G = X^T X (each
core computes its shard's partial; one fp16 AllReduce, overlapped with
the main stream), and only the centered remainder R needs the N^2 D
matmul -- it quantizes to fp8 ~10x better than P, so BOTH big matmuls
(scores and R V) run fp8 DoubleRow at 2x PE rate.  fp8 score errors only
perturb the second-order term (p-1)*dl, since the linear term is exact
through G -- the scheme self-corrects the linear part of all score
quantization noise, which is why the scores need no hi/lo residual pass.
Denominators: s = mu*N + q~ u/256 + sum_k r, with u = X^T 1 riding along
in the Gram AllReduce as 8 extra lhsT ones-rows.

Engine schedule: each score tile costs ScalarE TWO passes (exp and the
linear map t = l + mu feeding R = p - t), which exceeds the PE's fp8
issue rate -- so score groups of pair n are source-interleaved with the
R V groups of pair n-1, keeping PE busy while ScalarE drains.

Measured: rel_err ~6.8e-3 (vs 2e-2 gate; fp16-PV baseline was 1.455e-2).
"""

import sys

import numpy as np

try:
    import concourse.bass as bass  # noqa: F401
except ImportError:  # pragma: no cover
    sys.path.insert(0, "/opt/trn_rl_repo")

import concourse.bacc as bacc
import concourse.mybir as mybir
import concourse.tile as tile
from concourse.masks import make_identity
from concourse import bass_utils

N_TOKENS = 8192
EMBED = 1024
NCORES = 8
M = N_TOKENS // NCORES  # rows per core (1024)
P = 128  # partitions
DC = EMBED // P  # contraction chunks (8)
DE = EMBED + 8  # x-ext width: embed + 8 ones-columns (u rows in G)
NB = 512  # key-block width
NNB = N_TOKENS // NB  # key blocks (16)
MB = M // P  # query row-blocks per core (8)
VC = NB // P  # value chunks per key block (4)
GR = DE  # Gram rows (1032: 1024 dims + 8 u-rows)
FP32 = mybir.dt.float32
BF16 = mybir.dt.bfloat16
FP16 = mybir.dt.float16
FP8 = mybir.dt.float8e4
EXP = mybir.ActivationFunctionType.Exp
IDN = mybir.ActivationFunctionType.Identity
ADD = mybir.AluOpType.add
SUB = mybir.AluOpType.subtract
DROW = mybir.MatmulPerfMode.DoubleRow
# logits scale: 1/sqrt(EMBED) softmax scale x 1/8 undoing the 8*Z prescale
SCALE = 1.0 / 256.0
MU = 1.088  # centering constant 1 + E[r(l)], l ~ N(0, 0.41^2)


def _build():
    nc = bacc.Bacc(
        "TRN2", target_bir_lowering=False, debug=False, num_devices=NCORES
    )
    xt_shard = nc.dram_tensor("xt_shard", [EMBED, M], FP16, kind="ExternalInput").ap()
    xs_shard = nc.dram_tensor("xs_shard", [M, DE], FP16, kind="ExternalInput").ap()
    xt8_full = nc.dram_tensor(
        "xt8_full", [EMBED, N_TOKENS], FP8, kind="ExternalInput"
    ).ap()
    xv8_full = nc.dram_tensor(
        "xv8_full", [N_TOKENS, EMBED], FP8, kind="ExternalInput"
    ).ap()
    z_d = nc.dram_tensor("z", [EMBED, EMBED], FP16, kind="ExternalInput").ap()
    out_d = nc.dram_tensor("out", [M, EMBED], BF16, kind="ExternalOutput").ap()

    z_r = z_d.rearrange("(a p) d -> a p d", p=P)  # [DC, P, EMBED]
    xt_r = xt_shard.rearrange("(a p) m -> a p m", p=P)  # [DC, P, M]
    xs_r = xs_shard.rearrange("(a p) d -> a p d", p=P)  # [DC, P, DE]
    xt8_r = xt8_full.rearrange("(a p) n -> a p n", p=P)  # [DC, P, N]
    xv_r = xv8_full.rearrange("(t p) d -> t p d", p=P)  # [64, P, EMBED]
    out_r = out_d.rearrange("(t p) d -> t p d", p=P)  # [MB, P, EMBED]

    with tile.TileContext(nc) as tc:
        with (
            tc.tile_pool(name="persist", bufs=1) as pers,
            tc.tile_pool(name="gdram", bufs=1, space="DRAM") as dpool,
        ):
            ones8 = pers.tile([P, 2 * P], FP8)
            nc.vector.memset(ones8[:], 1.0)
            onesu = pers.tile([P, P], FP16)
            nc.vector.memset(onesu[:], MU / 8.0)
            mu_t = pers.tile([P, 1], FP32)
            nc.vector.memset(mu_t[:], MU)
            warm = pers.tile([P, 2], FP32)
            nc.vector.memset(warm[:], 0.0)
            # prime the ScalarE activation table while the head DMAs run
            nc.scalar.activation(out=warm[:, 1:2], in_=warm[:, 0:1],
                                 func=EXP, scale=SCALE)
            ident = pers.tile([P, P], FP32)
            make_identity(nc, ident[:])
            g8f = pers.tile([P, EMBED], FP32)  # u-rows staged fp32 (8 parts)
            # q~^T fp8 (scores stationary operand) and q~^T/256 fp16 (LV)
            qt8 = pers.tile([P, DC * M], FP8)
            qt16 = pers.tile([P, DC * M], FP16)
            # fp32 accumulator per query block: [p, mb*EMBED + dv]
            out_acc = pers.tile([P, MB * EMBED], FP32)
            # sum_k r contributions, replicated across partitions: [p, m]
            sums_acc = pers.tile([P, M], FP32)
            # AllReduce'd Gram: rows 0..1023 chunked, u-rows 1024..1031
            gsc = pers.tile([P, DC * EMBED], FP16)
            g8 = pers.tile([P, EMBED], FP16)
            # u along partitions (8 replicated cols per chunk), mu*u bcast
            ucol = pers.tile([P, DC * 8], FP16)
            ubc = pers.tile([P, EMBED], FP16)
            rsl = pers.tile([P, MB], FP32)
            g_in = dpool.tile([GR, EMBED], FP16)
            g_out = dpool.tile([GR, EMBED], FP16)

            # ---- Phase A: project q~^T = (8Z)^T @ x_own^T  (fp16)
            #      Phase G: partial Gram  [X|1]^T X          (fp16)
            with (
                tc.tile_pool(name="proj", bufs=1) as proj,
                tc.tile_pool(name="gst", bufs=4) as gst,
                tc.tile_pool(name="proj_ps", bufs=4, space="PSUM") as proj_ps,
            ):
                z_sb = proj.tile([P, DC * EMBED], FP16)
                xt_sb = proj.tile([P, DC * M], FP16)
                xs_sb = proj.tile([P, DC * DE], FP16)
                for a in range(DC):
                    nc.sync.dma_start(
                        out=z_sb[:, a * EMBED : (a + 1) * EMBED], in_=z_r[a]
                    )
                    nc.sync.dma_start(
                        out=xt_sb[:, a * M : (a + 1) * M], in_=xt_r[a]
                    )
                for a in range(DC):
                    nc.sync.dma_start(
                        out=xs_sb[:, a * DE : (a + 1) * DE], in_=xs_r[a]
                    )
                for j in range(M // NB):  # row half (j-outer: scores h=0
                    # needs every b of the j=0 half first)
                    for b in range(DC):  # output dim chunk
                        ps = proj_ps.tile([P, NB], FP32, tag="proj_ps")
                        for a in range(DC):  # contraction chunk
                            nc.tensor.matmul(
                                ps[:],
                                lhsT=z_sb[:, a * EMBED + b * P : a * EMBED + (b + 1) * P],
                                rhs=xt_sb[:, a * M + j * NB : a * M + (j + 1) * NB],
                                start=(a == 0),
                                stop=(a == DC - 1),
                            )
                        sl = slice(b * M + j * NB, b * M + (j + 1) * NB)
                        nc.vector.tensor_copy(out=qt8[:, sl], in_=ps[:])
                        nc.vector.tensor_scalar_mul(qt16[:, sl], ps[:], SCALE)

                # partial Gram on this core's rows: [X|1]^T X, fp16 out via
                # small staging tiles straight to DRAM for the AllReduce
                for pc in range(9):  # output partition chunk (8x128 + 1x8)
                    pw = P if pc < 8 else 8
                    for fh in range(2):
                        f0 = fh * NB
                        ps = proj_ps.tile([P, NB], FP32, tag="proj_ps")
                        for a in range(DC):
                            nc.tensor.matmul(
                                ps[:pw, :],
                                lhsT=xs_sb[:, a * DE + pc * P : a * DE + pc * P + pw],
                                rhs=xs_sb[:, a * DE + f0 : a * DE + f0 + NB],
                                start=(a == 0),
                                stop=(a == DC - 1),
                            )
                        gtile = gst.tile([P, NB], FP16, tag="gst")
                        nc.vector.tensor_copy(out=gtile[:pw, :], in_=ps[:pw, :])
                        nc.sync.dma_start(
                            out=g_in[pc * P : pc * P + pw, f0 : f0 + NB],
                            in_=gtile[:pw, :],
                        )

            # one fp16 AllReduce carries G and u; overlaps the score stream
            nc.gpsimd.collective_compute(
                "AllReduce",
                mybir.AluOpType.add,
                replica_groups=[list(range(NCORES))],
                ins=[g_in[:].opt()],
                outs=[g_out[:].opt()],
            )
            for a in range(DC):
                nc.sync.dma_start(
                    out=gsc[:, a * EMBED : (a + 1) * EMBED],
                    in_=g_out[a * P : (a + 1) * P, :],
                )
            nc.sync.dma_start(out=g8[:8, :], in_=g_out[EMBED:GR, :])

            # ---- Phase B: streaming pass over the 16 key blocks.
            # Source-interleaved software pipeline: score groups of pair n
            # alternate with R V groups of pair n-1.
            with (
                tc.tile_pool(name="kv", bufs=3) as kvp,
                tc.tile_pool(name="rb", bufs=2) as rbp,
                tc.tile_pool(name="ex", bufs=6) as exp_,
                tc.tile_pool(name="ps_s", bufs=4, space="PSUM") as ps_sp,
                tc.tile_pool(name="ps_u", bufs=2, space="PSUM") as ps_up,
                tc.tile_pool(name="ps_o", bufs=2, space="PSUM") as ps_op,
                tc.tile_pool(name="fin", bufs=2) as fin,
            ):
                scol = fin.tile([P, MB], FP32)
                rtot = fin.tile([P, MB], FP32)
                ones2_v = ones8[:].rearrange("p (s q) -> p s q", s=2)
                qh_v = qt8[:].rearrange("p (b m) -> p b m", b=DC)  # [P, DC, M]

                def rv_group(rts, vts, np_, mb, h):
                    ps_o = ps_op.tile([P, NB], FP32, tag="ps_o")
                    for blk in range(2):
                        r_v = rts[blk][:].rearrange("p (c m) -> p c m", c=VC)
                        v_v = vts[blk][:].rearrange("p (t e) -> p t e", t=VC)
                        for t2 in range(VC // 2):
                            nc.tensor.matmul(
                                ps_o[:],
                                lhsT=r_v[:, 2 * t2 : 2 * t2 + 2, mb * P : (mb + 1) * P],
                                rhs=v_v[:, 2 * t2 : 2 * t2 + 2, h * NB : (h + 1) * NB],
                                start=(blk == 0 and t2 == 0),
                                stop=(blk == 1 and t2 == VC // 2 - 1),
                                perf_mode=DROW,
                            )
                    dst = out_acc[:, mb * EMBED + h * NB : mb * EMBED + (h + 1) * NB]
                    if np_ == 0:
                        nc.vector.tensor_copy(out=dst, in_=ps_o[:])
                    else:
                        nc.vector.tensor_tensor(out=dst, in0=dst, in1=ps_o[:], op=ADD)

                def sums_group(rts, np_, h):
                    ps_sum = ps_up.tile([P, NB], FP32, tag="ps_sum")
                    for blk in range(2):
                        r_v = rts[blk][:].rearrange("p (c m) -> p c m", c=VC)
                        for cc in range(VC // 2):
                            nc.tensor.matmul(
                                ps_sum[:],
                                lhsT=ones2_v,
                                rhs=r_v[:, 2 * cc : 2 * cc + 2, h * NB : (h + 1) * NB],
                                start=(blk == 0 and cc == 0),
                                stop=(blk == 1 and cc == VC // 2 - 1),
                                perf_mode=DROW,
                            )
                    dsts = sums_acc[:, h * NB : (h + 1) * NB]
                    if np_ == 0:
                        nc.vector.tensor_copy(out=dsts, in_=ps_sum[:])
                    else:
                        nc.vector.tensor_tensor(
                            out=dsts, in0=dsts, in1=ps_sum[:], op=ADD
                        )

                prev = None  # (rts, vts, np_) of previous pair
                for np_ in range(NNB // 2):
                    rts, vts = [], []
                    kt_vs = []
                    for blk in range(2):
                        nb = 2 * np_ + blk
                        vtile = kvp.tile([P, VC * EMBED], FP8, tag=f"vt{blk}")
                        for c in range(VC):
                            nc.sync.dma_start(
                                out=vtile[:, c * EMBED : (c + 1) * EMBED],
                                in_=xv_r[nb * VC + c],
                            )
                        ktile = kvp.tile([P, DC * NB], FP8, tag=f"kt{blk}")
                        for b in range(DC):
                            nc.sync.dma_start(
                                out=ktile[:, b * NB : (b + 1) * NB],
                                in_=xt8_r[b, :, nb * NB : (nb + 1) * NB],
                            )
                        kt_vs.append(ktile[:].rearrange("p (b n) -> p b n", b=DC))
                        rtile = rbp.tile([P, VC * M], FP8, tag=f"rt{blk}")
                        rts.append(rtile)
                        vts.append(vtile)

                    # interleave: 16 score groups with prev pair's RV groups
                    fill = []
                    if prev is not None:
                        prts, pvts, pnp = prev
                        fill = [
                            (mb, h) for mb in range(MB) for h in range(EMBED // NB)
                        ]
                    fi = 0
                    for blk in range(2):
                        for h in range(M // NB):  # query column half
                            for c in range(VC):  # key chunk within block
                                ps_s = ps_sp.tile([P, NB], FP32, tag="ps_s")
                                for bb in range(DC // 2):
                                    nc.tensor.matmul(
                                        ps_s[:],
                                        lhsT=kt_vs[blk][
                                            :, 2 * bb : 2 * bb + 2, c * P : (c + 1) * P
                                        ],
                                        rhs=qh_v[
                                            :, 2 * bb : 2 * bb + 2,
                                            h * NB : (h + 1) * NB,
                                        ],
                                        start=(bb == 0),
                                        stop=(bb == DC // 2 - 1),
                                        perf_mode=DROW,
                                    )
                                csl = slice(c * M + h * NB, c * M + (h + 1) * NB)
                                pe = exp_.tile([P, NB], FP16, tag="pe")
                                nc.scalar.activation(
                                    out=pe[:], in_=ps_s[:], func=EXP, scale=SCALE
                                )
                                tl = exp_.tile([P, NB], FP16, tag="tl")
                                nc.vector.tensor_scalar(
                                    out=tl[:], in0=ps_s[:],
                                    scalar1=SCALE, scalar2=MU,
                                    op0=mybir.AluOpType.mult, op1=ADD,
                                )
                                nc.vector.tensor_tensor(
                                    out=rts[blk][:, csl], in0=pe[:], in1=tl[:], op=SUB
                                )
                                if fi < len(fill):
                                    rv_group(prts, pvts, pnp, *fill[fi])
                                    fi += 1
                    if prev is not None:
                        for h in range(M // NB):
                            sums_group(prts, pnp, h)

                    if np_ == 5:
                        # AllReduce long done: exact linear terms while the
                        # stream continues.  u-cols from transposed u-rows.
                        nc.vector.tensor_copy(out=g8f[:8, :], in_=g8[:8, :])
                        for a in range(DC):
                            ps_t = ps_up.tile([P, NB], FP32, tag="ps_sum")
                            nc.tensor.transpose(
                                out=ps_t[:, 0:8],
                                in_=g8f[:8, a * P : (a + 1) * P],
                                identity=ident[:8, :8],
                            )
                            nc.vector.tensor_copy(
                                out=ucol[:, a * 8 : (a + 1) * 8], in_=ps_t[:, 0:8]
                            )
                        for mb in range(MB):
                            for fh in range(EMBED // NB):
                                ps_l = ps_op.tile([P, NB], FP32, tag="ps_o")
                                for a in range(DC):
                                    nc.tensor.matmul(
                                        ps_l[:],
                                        lhsT=qt16[:, a * M + mb * P : a * M + (mb + 1) * P],
                                        rhs=gsc[:, a * EMBED + fh * NB : a * EMBED + (fh + 1) * NB],
                                        start=(a == 0),
                                        stop=(a == DC - 1),
                                    )
                                dstl = out_acc[
                                    :, mb * EMBED + fh * NB : mb * EMBED + (fh + 1) * NB
                                ]
                                nc.vector.tensor_tensor(
                                    out=dstl, in0=dstl, in1=ps_l[:], op=ADD
                                )
                            ps_r = ps_up.tile([P, NB], FP32, tag="ps_sum")
                            for a in range(DC):
                                nc.tensor.matmul(
                                    ps_r[:, :8],
                                    lhsT=qt16[:, a * M + mb * P : a * M + (mb + 1) * P],
                                    rhs=ucol[:, a * 8 : (a + 1) * 8],
                                    start=(a == 0),
                                    stop=(a == DC - 1),
                                )
                            nc.vector.tensor_copy(
                                out=rsl[:, mb : mb + 1], in_=ps_r[:, 0:1]
                            )
                        for fh in range(EMBED // NB):
                            ps_b = ps_op.tile([P, NB], FP32, tag="ps_o")
                            nc.tensor.matmul(
                                ps_b[:],
                                lhsT=onesu[:8, :],
                                rhs=g8[:8, fh * NB : (fh + 1) * NB],
                                start=True,
                                stop=True,
                            )
                            nc.vector.tensor_copy(
                                out=ubc[:, fh * NB : (fh + 1) * NB], in_=ps_b[:]
                            )
                        for mb in range(MB):
                            dstu = out_acc[:, mb * EMBED : (mb + 1) * EMBED]
                            nc.vector.tensor_tensor(
                                out=dstu, in0=dstu, in1=ubc[:], op=ADD
                            )

                    prev = (rts, vts, np_)

                # tail: last pair's sums, denominators, then RV + divide
                prts, pvts, pnp = prev
                for h in range(M // NB):
                    sums_group(prts, pnp, h)
                for mb in range(MB):
                    ps_f = ps_up.tile([P, NB], FP32, tag="ps_sum")
                    nc.tensor.transpose(
                        out=ps_f[:, 0:P],
                        in_=sums_acc[:, mb * P : (mb + 1) * P],
                        identity=ident[:],
                    )
                    nc.vector.tensor_copy(
                        out=scol[:, mb : mb + 1], in_=ps_f[:, 0:1]
                    )
                nc.vector.tensor_tensor(out=scol[:], in0=scol[:], in1=rsl[:], op=ADD)
                nc.vector.tensor_scalar_add(scol[:], scol[:], MU * N_TOKENS)
                nc.vector.reciprocal(out=rtot[:], in_=scol[:])
                for mb in range(MB):
                    for h in range(EMBED // NB):
                        rv_group(prts, pvts, pnp, mb, h)
                    outf = fin.tile([P, EMBED], BF16, tag="outf")
                    nc.vector.tensor_scalar_mul(
                        outf[:],
                        out_acc[:, mb * EMBED : (mb + 1) * EMBED],
                        rtot[:, mb : mb + 1],
                    )
                    nc.sync.dma_start(out=out_r[mb], in_=outf[:])

    nc.compile()
    return nc


_NC = None


def _get_nc():
    global _NC
    if _NC is None:
        _NC = _build()
    return _NC


def _run(x, rotation_params, entangle_params, **spmd_kwargs):
    x = np.ascontiguousarray(np.asarray(x, dtype=np.float32))
    wq = np.asarray(rotation_params, dtype=np.float32).reshape(EMBED, EMBED)
    wk = np.asarray(entangle_params, dtype=np.float32).reshape(EMBED, EMBED)
    import ml_dtypes

    # offline weight folding: Z = 8 * Wq Wk^T (the 8x keeps the fp8 q~ in
    # e4m3's normal range; undone in the exp/linear scales)
    z8 = (8.0 * (wq @ wk.T)).astype(np.float16)
    xt = np.ascontiguousarray(x.T)
    xt16 = xt.astype(np.float16)
    xt8 = xt.astype(ml_dtypes.float8_e4m3)
    xv8 = x.astype(ml_dtypes.float8_e4m3)
    x16 = x.astype(np.float16)
    ones8c = np.ones((M, 8), np.float16)
    in_maps = [
        {
            "xt_shard": np.ascontiguousarray(xt16[:, i * M : (i + 1) * M]),
            "xs_shard": np.ascontiguousarray(
                np.concatenate([x16[i * M : (i + 1) * M], ones8c], axis=1)
            ),
            "xt8_full": xt8,
            "xv8_full": xv8,
            "z": z8,
        }
        for i in range(NCORES)
    ]
    res = bass_utils.run_bass_kernel_spmd(
        _get_nc(), in_maps, core_ids=list(range(NCORES)), **spmd_kwargs
    )
    out = np.concatenate(
        [res.results[i]["out"].astype(np.float32) for i in range(NCORES)], axis=0
    )
    return out, res


def kernel(x, rotation_params, entangle_params):
    out, _ = _run(x, rotation_params, entangle_params)
    return out


# revision 15
# speedup vs baseline: 1.0851x; 1.0851x over previous
"""Trainium2 Bass kernel for ClassicalSelfAttention.

  out = softmax((x @ Wq) @ (x @ Wk)^T / sqrt(D)) @ x      x: [8192, 1024] f32

Algebraic restructuring 1 (weight folding): scores = x (Wq Wk^T) x^T; the
weight matrices fold offline into Z = Wq Wk^T.  Each core projects only
its own row-shard (q~ = x_own @ 8Z) and computes its scores row-block
directly against x^T streamed in fp8.  No K projection, no K^T AllGather.

Algebraic restructuring 2 (LINEAR SPLIT): the logits are small
(l ~ N(0, 0.41^2)), so exp(l) = 1 + l + r(l) with the curvature
remainder r = e^l - 1 - l tiny (rms ~0.12 vs ~1.18 for e^l).  The
attention numerator splits exactly:

   P V = mu * colsum(V)  +  q~ (X^T X) / 256  +  R V

The linear term collapses through the D x D Gram matrix G = X^T X (each
core computes its shard's partial; one fp16 AllReduce, overlapped with
the main stream), and only the centered remainder R needs the N^2 D
matmul -- it quantizes to fp8 ~10x better than P, so BOTH big matmuls
(scores and R V) run fp8 DoubleRow at 2x-per-instruction PE rate.  fp8
score errors only perturb the second-order term (p-1)*dl, since the
linear term is exact through G -- the scheme self-corrects the linear
part of all score quantization noise, which is why the scores need no
hi/lo residual pass.  Denominators: s = mu*N + q~ u/256 + sum_k r, with
u = X^T 1 riding along in the Gram AllReduce as 8 extra lhsT ones-rows.

Engine schedule: each score tile costs ScalarE TWO passes (exp and the
linear map t = l + mu feeding R = p - t), which exceeds the PE's fp8
issue rate -- so score groups of pair n are source-interleaved with the
R V groups of pair n-1, and pair 0's slack is filled with the Gram
matmuls.  All AllReduce bounce DMAs ride the gpsimd queue so the
collective-gated read-back never blocks the K/V stream on the sync
queues.  Note the PE downclocks 2.4->2.0 GHz under the sustained fp8
DoubleRow power draw (P0), so DoubleRow nets ~1.6x, not 2x.

Measured: rel_err ~6.8e-3 (vs 2e-2 gate; fp16-PV baseline was 1.455e-2).
"""

import sys

import numpy as np

try:
    import concourse.bass as bass  # noqa: F401
except ImportError:  # pragma: no cover
    sys.path.insert(0, "/opt/trn_rl_repo")

import concourse.bacc as bacc
import concourse.mybir as mybir
import concourse.tile as tile
from concourse.masks import make_identity
from concourse import bass_utils

N_TOKENS = 8192
EMBED = 1024
NCORES = 8
M = N_TOKENS // NCORES  # rows per core (1024)
P = 128  # partitions
DC = EMBED // P  # contraction chunks (8)
DE = EMBED + 8  # x-ext width: embed + 8 ones-columns (u rows in G)
NB = 512  # key-block width
NNB = N_TOKENS // NB  # key blocks (16)
MB = M // P  # query row-blocks per core (8)
VC = NB // P  # value chunks per key block (4)
GR = DE  # Gram rows (1032: 1024 dims + 8 u-rows)
FP32 = mybir.dt.float32
BF16 = mybir.dt.bfloat16
FP16 = mybir.dt.float16
FP8 = mybir.dt.float8e4
EXP = mybir.ActivationFunctionType.Exp
IDN = mybir.ActivationFunctionType.Identity
ADD = mybir.AluOpType.add
SUB = mybir.AluOpType.subtract
DROW = mybir.MatmulPerfMode.DoubleRow
# logits scale: 1/sqrt(EMBED) softmax scale x 1/8 undoing the 8*Z prescale
SCALE = 1.0 / 256.0
MU = 1.088  # centering constant 1 + E[r(l)], l ~ N(0, 0.41^2)


def _build():
    nc = bacc.Bacc(
        "TRN2", target_bir_lowering=False, debug=False, num_devices=NCORES
    )
    xt_shard = nc.dram_tensor("xt_shard", [EMBED, M], FP16, kind="ExternalInput").ap()
    xs_shard = nc.dram_tensor("xs_shard", [M, DE], FP16, kind="ExternalInput").ap()
    xt8_full = nc.dram_tensor(
        "xt8_full", [EMBED, N_TOKENS], FP8, kind="ExternalInput"
    ).ap()
    xv8_full = nc.dram_tensor(
        "xv8_full", [N_TOKENS, EMBED], FP8, kind="ExternalInput"
    ).ap()
    z_d = nc.dram_tensor("z", [EMBED, EMBED], FP16, kind="ExternalInput").ap()
    out_d = nc.dram_tensor("out", [M, EMBED], BF16, kind="ExternalOutput").ap()

    z_r = z_d.rearrange("(a p) d -> a p d", p=P)  # [DC, P, EMBED]
    xt_r = xt_shard.rearrange("(a p) m -> a p m", p=P)  # [DC, P, M]
    xs_r = xs_shard.rearrange("(a p) d -> a p d", p=P)  # [DC, P, DE]
    xt8_r = xt8_full.rearrange("(a p) n -> a p n", p=P)  # [DC, P, N]
    xv_r = xv8_full.rearrange("(t p) d -> t p d", p=P)  # [64, P, EMBED]
    out_r = out_d.rearrange("(t p) d -> t p d", p=P)  # [MB, P, EMBED]

    with tile.TileContext(nc) as tc:
        with (
            tc.tile_pool(name="persist", bufs=1) as pers,
            tc.tile_pool(name="gdram", bufs=1, space="DRAM") as dpool,
            tc.tile_pool(name="gx", bufs=1) as gxp,
            tc.tile_pool(name="gst", bufs=4) as gst,
        ):
            ones8 = pers.tile([P, 2 * P], FP8)
            nc.vector.memset(ones8[:], 1.0)
            onesu = pers.tile([P, P], FP16)
            nc.vector.memset(onesu[:], MU / 8.0)
            mu_t = pers.tile([P, 1], FP32)
            nc.vector.memset(mu_t[:], MU)
            warm = pers.tile([P, 2], FP32)
            nc.vector.memset(warm[:], 0.0)
            # prime the ScalarE activation table while the head DMAs run
            nc.scalar.activation(out=warm[:, 1:2], in_=warm[:, 0:1],
                                 func=EXP, scale=SCALE)
            ident = pers.tile([P, P], FP32)
            make_identity(nc, ident[:])
            g8f = pers.tile([P, EMBED], FP32)  # u-rows staged fp32 (8 parts)
            # q~^T fp8 (scores stationary operand) and q~^T/256 fp16 (LV)
            qt8 = pers.tile([P, DC * M], FP8)
            qt16 = pers.tile([P, DC * M], FP16)
            # fp32 accumulator per query block: [p, mb*EMBED + dv]
            out_acc = pers.tile([P, MB * EMBED], FP32)
            # sum_k r contributions, replicated across partitions: [p, m]
            sums_acc = pers.tile([P, M], FP32)
            # AllReduce'd Gram: rows 0..1023 chunked, u-rows 1024..1031
            gsc = pers.tile([P, DC * EMBED], FP16)
            g8 = pers.tile([P, EMBED], FP16)
            # u along partitions (8 replicated cols per chunk), mu*u bcast
            ucol = pers.tile([P, DC * 8], FP16)
            ubc = pers.tile([P, EMBED], FP16)
            rsl = pers.tile([P, MB], FP32)
            g_in = dpool.tile([GR, EMBED], FP16)
            g_out = dpool.tile([GR, EMBED], FP16)
            xs_sb = gxp.tile([P, DC * DE], FP16)

            def g_group(pc, fh, psp, pstag):
                pw = P if pc < 8 else 8
                f0 = fh * NB
                ps = psp.tile([P, NB], FP32, tag=pstag, name=f"gps{pc}_{fh}")
                for a in range(DC):
                    nc.tensor.matmul(
                        ps[:pw, :],
                        lhsT=xs_sb[:, a * DE + pc * P : a * DE + pc * P + pw],
                        rhs=xs_sb[:, a * DE + f0 : a * DE + f0 + NB],
                        start=(a == 0),
                        stop=(a == DC - 1),
                    )
                gtile = gst.tile([P, NB], FP16, tag="gst", name=f"gt{pc}_{fh}")
                nc.vector.tensor_copy(out=gtile[:pw, :], in_=ps[:pw, :])
                nc.gpsimd.dma_start(
                    out=g_in[pc * P : pc * P + pw, f0 : f0 + NB],
                    in_=gtile[:pw, :],
                )

            # ---- Phase A: project q~^T = (8Z)^T @ x_own^T  (fp16), plus
            # the first Gram chunks; the rest fill pair 0's interleave
            # slots in phase B (pair 0 has no previous-pair R V work and
            # its score stream alone is ScalarE-bound).
            with (
                tc.tile_pool(name="proj", bufs=1) as proj,
                tc.tile_pool(name="proj_ps", bufs=4, space="PSUM") as proj_ps,
            ):
                z_sb = proj.tile([P, DC * EMBED], FP16)
                xt_sb = proj.tile([P, DC * M], FP16)
                for a in range(DC):
                    nc.sync.dma_start(
                        out=z_sb[:, a * EMBED : (a + 1) * EMBED], in_=z_r[a]
                    )
                    nc.sync.dma_start(
                        out=xt_sb[:, a * M : a * M + NB], in_=xt_r[a][:, 0:NB]
                    )
                for a in range(DC):
                    nc.sync.dma_start(
                        out=xt_sb[:, a * M + NB : (a + 1) * M],
                        in_=xt_r[a][:, NB:M],
                    )
                    nc.sync.dma_start(
                        out=xs_sb[:, a * DE : (a + 1) * DE], in_=xs_r[a]
                    )
                for j in range(M // NB):  # row half (j-outer: scores h=0
                    # needs every b of the j=0 half first)
                    for b in range(DC):  # output dim chunk
                        ps = proj_ps.tile([P, NB], FP32, tag="proj_ps")
                        for a in range(DC):  # contraction chunk
                            nc.tensor.matmul(
                                ps[:],
                                lhsT=z_sb[:, a * EMBED + b * P : a * EMBED + (b + 1) * P],
                                rhs=xt_sb[:, a * M + j * NB : a * M + (j + 1) * NB],
                                start=(a == 0),
                                stop=(a == DC - 1),
                            )
                        sl = slice(b * M + j * NB, b * M + (j + 1) * NB)
                        nc.vector.tensor_copy(out=qt8[:, sl], in_=ps[:])
                        nc.vector.tensor_scalar_mul(qt16[:, sl], ps[:], SCALE)
                for pc in range(4):  # Gram head chunks
                    for fh in range(2):
                        g_group(pc, fh, proj_ps, "proj_ps")

            g_fill = [(pc, fh) for pc in range(4, 9) for fh in range(2)]

            # ---- Phase B: streaming pass over the 16 key blocks.
            # Source-interleaved software pipeline: score groups of pair n
            # alternate with R V groups of pair n-1 (Gram groups at pair 0).
            with (
                tc.tile_pool(name="kv", bufs=3) as kvp,
                tc.tile_pool(name="rb", bufs=2) as rbp,
                tc.tile_pool(name="ex", bufs=6) as exp_,
                tc.tile_pool(name="ps_s", bufs=4, space="PSUM") as ps_sp,
                tc.tile_pool(name="ps_u", bufs=2, space="PSUM") as ps_up,
                tc.tile_pool(name="ps_o", bufs=2, space="PSUM") as ps_op,
                tc.tile_pool(name="fin", bufs=2) as fin,
            ):
                scol = fin.tile([P, MB], FP32)
                rtot = fin.tile([P, MB], FP32)
                ones2_v = ones8[:].rearrange("p (s q) -> p s q", s=2)
                qh_v = qt8[:].rearrange("p (b m) -> p b m", b=DC)  # [P, DC, M]

                def rv_group(rts, vts, np_, mb, h):
                    ps_o = ps_op.tile([P, NB], FP32, tag="ps_o", name=f"rv{mb}_{h}")
                    for blk in range(2):
                        r_v = rts[blk][:].rearrange("p (c m) -> p c m", c=VC)
                        v_v = vts[blk][:].rearrange("p (t e) -> p t e", t=VC)
                        for t2 in range(VC // 2):
                            nc.tensor.matmul(
                                ps_o[:],
                                lhsT=r_v[:, 2 * t2 : 2 * t2 + 2, mb * P : (mb + 1) * P],
                                rhs=v_v[:, 2 * t2 : 2 * t2 + 2, h * NB : (h + 1) * NB],
                                start=(blk == 0 and t2 == 0),
                                stop=(blk == 1 and t2 == VC // 2 - 1),
                                perf_mode=DROW,
                            )
                    dst = out_acc[:, mb * EMBED + h * NB : mb * EMBED + (h + 1) * NB]
                    if np_ == 0:
                        nc.vector.tensor_copy(out=dst, in_=ps_o[:])
                    else:
                        nc.vector.tensor_tensor(out=dst, in0=dst, in1=ps_o[:], op=ADD)

                def sums_group(rts, np_, h):
                    ps_sum = ps_up.tile([P, NB], FP32, tag="ps_sum", name=f"su{h}")
                    for blk in range(2):
                        r_v = rts[blk][:].rearrange("p (c m) -> p c m", c=VC)
                        for cc in range(VC // 2):
                            nc.tensor.matmul(
                                ps_sum[:],
                                lhsT=ones2_v,
                                rhs=r_v[:, 2 * cc : 2 * cc + 2, h * NB : (h + 1) * NB],
                                start=(blk == 0 and cc == 0),
                                stop=(blk == 1 and cc == VC // 2 - 1),
                                perf_mode=DROW,
                            )
                    dsts = sums_acc[:, h * NB : (h + 1) * NB]
                    if np_ == 0:
                        nc.vector.tensor_copy(out=dsts, in_=ps_sum[:])
                    else:
                        nc.vector.tensor_tensor(
                            out=dsts, in0=dsts, in1=ps_sum[:], op=ADD
                        )

                prev = None  # (rts, vts, np_) of previous pair
                for np_ in range(NNB // 2):
                    rts, vts = [], []
                    kt_vs = []
                    for blk in range(2):
                        nb = 2 * np_ + blk
                        vtile = kvp.tile([P, VC * EMBED], FP8, tag=f"vt{blk}")
                        for c in range(VC):
                            nc.sync.dma_start(
                                out=vtile[:, c * EMBED : (c + 1) * EMBED],
                                in_=xv_r[nb * VC + c],
                            )
                        ktile = kvp.tile([P, DC * NB], FP8, tag=f"kt{blk}")
                        for b in range(DC):
                            nc.sync.dma_start(
                                out=ktile[:, b * NB : (b + 1) * NB],
                                in_=xt8_r[b, :, nb * NB : (nb + 1) * NB],
                            )
                        kt_vs.append(ktile[:].rearrange("p (b n) -> p b n", b=DC))
                        rtile = rbp.tile([P, VC * M], FP8, tag=f"rt{blk}")
                        rts.append(rtile)
                        vts.append(vtile)

                    # interleave: 16 score groups with fill work (prev RV,
                    # or Gram groups at pair 0)
                    if prev is not None:
                        prts, pvts, pnp = prev
                        fill = [
                            (lambda mb=mb, h=h: rv_group(prts, pvts, pnp, mb, h))
                            for mb in range(MB) for h in range(EMBED // NB)
                        ]
                    else:
                        fill = [
                            (lambda pc=pc, fh=fh: g_group(pc, fh, ps_op, "ps_o"))
                            for (pc, fh) in g_fill
                        ]
                    fi = 0
                    for blk in range(2):
                        for h in range(M // NB):  # query column half
                            for c in range(VC):  # key chunk within block
                                ps_s = ps_sp.tile([P, NB], FP32, tag="ps_s")
                                for bb in range(DC // 2):
                                    nc.tensor.matmul(
                                        ps_s[:],
                                        lhsT=kt_vs[blk][
                                            :, 2 * bb : 2 * bb + 2, c * P : (c + 1) * P
                                        ],
                                        rhs=qh_v[
                                            :, 2 * bb : 2 * bb + 2,
                                            h * NB : (h + 1) * NB,
                                        ],
                                        start=(bb == 0),
                                        stop=(bb == DC // 2 - 1),
                                        perf_mode=DROW,
                                    )
                                csl = slice(c * M + h * NB, c * M + (h + 1) * NB)
                                pe = exp_.tile([P, NB], FP16, tag="pe")
                                nc.scalar.activation(
                                    out=pe[:], in_=ps_s[:], func=EXP, scale=SCALE
                                )
                                tl = exp_.tile([P, NB], FP16, tag="tl")
                                nc.scalar.activation(
                                    out=tl[:], in_=ps_s[:], func=IDN,
                                    scale=SCALE, bias=mu_t[:],
                                )
                                nc.vector.tensor_tensor(
                                    out=rts[blk][:, csl], in0=pe[:], in1=tl[:], op=SUB
                                )
                                if fi < len(fill):
                                    fill[fi]()
                                    fi += 1
                    while fi < len(fill):
                        fill[fi]()
                        fi += 1
                    if prev is not None:
                        for h in range(M // NB):
                            sums_group(prts, pnp, h)

                    if np_ == 0:
                        # all Gram partials written: fire the AllReduce and
                        # pull the result back on the gpsimd queue (never
                        # blocks the sync-queue K/V stream)
                        nc.gpsimd.collective_compute(
                            "AllReduce",
                            mybir.AluOpType.add,
                            replica_groups=[list(range(NCORES))],
                            ins=[g_in[:].opt()],
                            outs=[g_out[:].opt()],
                        )
                        for a in range(DC):
                            nc.gpsimd.dma_start(
                                out=gsc[:, a * EMBED : (a + 1) * EMBED],
                                in_=g_out[a * P : (a + 1) * P, :],
                            )
                        nc.gpsimd.dma_start(out=g8[:8, :], in_=g_out[EMBED:GR, :])

                    if np_ == 5:
                        # AllReduce long done: exact linear terms while the
                        # stream continues.  u-cols from transposed u-rows.
                        nc.vector.tensor_copy(out=g8f[:8, :], in_=g8[:8, :])
                        for a in range(DC):
                            ps_t = ps_up.tile([P, NB], FP32, tag="ps_sum")
                            nc.tensor.transpose(
                                out=ps_t[:, 0:8],
                                in_=g8f[:8, a * P : (a + 1) * P],
                                identity=ident[:8, :8],
                            )
                            nc.vector.tensor_copy(
                                out=ucol[:, a * 8 : (a + 1) * 8], in_=ps_t[:, 0:8]
                            )
                        for mb in range(MB):
                            for fh in range(EMBED // NB):
                                ps_l = ps_op.tile([P, NB], FP32, tag="ps_o")
                                for a in range(DC):
                                    nc.tensor.matmul(
                                        ps_l[:],
                                        lhsT=qt16[:, a * M + mb * P : a * M + (mb + 1) * P],
                                        rhs=gsc[:, a * EMBED + fh * NB : a * EMBED + (fh + 1) * NB],
                                        start=(a == 0),
                                        stop=(a == DC - 1),
                                    )
                                dstl = out_acc[
                                    :, mb * EMBED + fh * NB : mb * EMBED + (fh + 1) * NB
                                ]
                                nc.vector.tensor_tensor(
                                    out=dstl, in0=dstl, in1=ps_l[:], op=ADD
                                )
                            ps_r = ps_up.tile([P, NB], FP32, tag="ps_sum")
                            for a in range(DC):
                                nc.tensor.matmul(
                                    ps_r[:, :8],
                                    lhsT=qt16[:, a * M + mb * P : a * M + (mb + 1) * P],
                                    rhs=ucol[:, a * 8 : (a + 1) * 8],
                                    start=(a == 0),
                                    stop=(a == DC - 1),
                                )
                            nc.vector.tensor_copy(
                                out=rsl[:, mb : mb + 1], in_=ps_r[:, 0:1]
                            )
                        for fh in range(EMBED // NB):
                            ps_b = ps_op.tile([P, NB], FP32, tag="ps_o")
                            nc.tensor.matmul(
                                ps_b[:],
                                lhsT=onesu[:8, :],
                                rhs=g8[:8, fh * NB : (fh + 1) * NB],
                                start=True,
                                stop=True,
                            )
                            nc.vector.tensor_copy(
                                out=ubc[:, fh * NB : (fh + 1) * NB], in_=ps_b[:]
                            )
                        for mb in range(MB):
                            dstu = out_acc[:, mb * EMBED : (mb + 1) * EMBED]
                            nc.vector.tensor_tensor(
                                out=dstu, in0=dstu, in1=ubc[:], op=ADD
                            )

                    prev = (rts, vts, np_)

                # tail: last pair's sums, denominators, then RV + divide
                prts, pvts, pnp = prev
                for h in range(M // NB):
                    sums_group(prts, pnp, h)
                for mb in range(MB):
                    ps_f = ps_up.tile([P, NB], FP32, tag="ps_sum")
                    nc.tensor.transpose(
                        out=ps_f[:, 0:P],
                        in_=sums_acc[:, mb * P : (mb + 1) * P],
                        identity=ident[:],
                    )
                    nc.vector.tensor_copy(
                        out=scol[:, mb : mb + 1], in_=ps_f[:, 0:1]
                    )
                nc.vector.tensor_tensor(out=scol[:], in0=scol[:], in1=rsl[:], op=ADD)
                nc.vector.tensor_scalar_add(scol[:], scol[:], MU * N_TOKENS)
                nc.vector.reciprocal(out=rtot[:], in_=scol[:])
                for mb in range(MB):
                    for h in range(EMBED // NB):
                        rv_group(prts, pvts, pnp, mb, h)
                    outf = fin.tile([P, EMBED], BF16, tag="outf")
                    nc.vector.tensor_scalar_mul(
                        outf[:],
                        out_acc[:, mb * EMBED : (mb + 1) * EMBED],
                        rtot[:, mb : mb + 1],
                    )
                    nc.sync.dma_start(out=out_r[mb], in_=outf[:])

    nc.compile()
    return nc


_NC = None


def _get_nc():
    global _NC
    if _NC is None:
        _NC = _build()
    return _NC


def _run(x, rotation_params, entangle_params, **spmd_kwargs):
    x = np.ascontiguousarray(np.asarray(x, dtype=np.float32))
    wq = np.asarray(rotation_params, dtype=np.float32).reshape(EMBED, EMBED)
    wk = np.asarray(entangle_params, dtype=np.float32).reshape(EMBED, EMBED)
    import ml_dtypes

    # offline weight folding: Z = 8 * Wq Wk^T (the 8x keeps the fp8 q~ in
    # e4m3's normal range; undone in the exp/linear scales)
    z8 = (8.0 * (wq @ wk.T)).astype(np.float16)
    xt = np.ascontiguousarray(x.T)
    xt16 = xt.astype(np.float16)
    xt8 = xt.astype(ml_dtypes.float8_e4m3)
    xv8 = x.astype(ml_dtypes.float8_e4m3)
    x16 = x.astype(np.float16)
    ones8c = np.ones((M, 8), np.float16)
    in_maps = [
        {
            "xt_shard": np.ascontiguousarray(xt16[:, i * M : (i + 1) * M]),
            "xs_shard": np.ascontiguousarray(
                np.concatenate([x16[i * M : (i + 1) * M], ones8c], axis=1)
            ),
            "xt8_full": xt8,
            "xv8_full": xv8,
            "z": z8,
        }
        for i in range(NCORES)
    ]
    res = bass_utils.run_bass_kernel_spmd(
        _get_nc(), in_maps, core_ids=list(range(NCORES)), **spmd_kwargs
    )
    out = np.concatenate(
        [res.results[i]["out"].astype(np.float32) for i in range(NCORES)], axis=0
    )
    return out, res


def kernel(x, rotation_params, entangle_params):
    out, _ = _run(x, rotation_params, entangle_params)
    return out


# revision 19
# speedup vs baseline: 1.0894x; 1.0039x over previous
"""Trainium2 Bass kernel for ClassicalSelfAttention.

  out = softmax((x @ Wq) @ (x @ Wk)^T / sqrt(D)) @ x      x: [8192, 1024] f32

Algebraic restructuring 1 (weight folding): scores = x (Wq Wk^T) x^T; the
weight matrices fold offline into Z = Wq Wk^T.  Each core projects only
its own row-shard (q~ = x_own @ 8Z) and computes its scores row-block
directly against x^T streamed in fp8.  No K projection, no K^T AllGather.

Algebraic restructuring 2 (LINEAR SPLIT): the logits are small
(l ~ N(0, 0.41^2)), so exp(l) = 1 + l + r(l) with the curvature
remainder r = e^l - 1 - l tiny (rms ~0.12 vs ~1.18 for e^l).  The
attention numerator splits exactly:

   P V = mu * colsum(V)  +  q~ (X^T X) / 256  +  R V

The linear term collapses through the D x D Gram matrix G = X^T X (each
core computes its shard's partial; one fp16 AllReduce, overlapped with
the main stream), and only the centered remainder R needs the N^2 D
matmul -- it quantizes to fp8 ~10x better than P, so BOTH big matmuls
(scores and R V) run fp8 DoubleRow at 2x-per-instruction PE rate.  fp8
score errors only perturb the second-order term (p-1)*dl, since the
linear term is exact through G -- the scheme self-corrects the linear
part of all score quantization noise, which is why the scores need no
hi/lo residual pass.  Denominators: s = mu*N + q~ u/256 + sum_k r, with
u = X^T 1 riding along in the Gram AllReduce as 8 extra lhsT ones-rows.

Engine schedule: each score tile costs ScalarE TWO passes (exp and the
linear map t = l + mu feeding R = p - t), which exceeds the PE's fp8
issue rate -- so score groups of pair n are source-interleaved with the
R V groups of pair n-1, and pair 0's slack is filled with the Gram
matmuls.  All AllReduce bounce DMAs ride the gpsimd queue so the
collective-gated read-back never blocks the K/V stream on the sync
queues.  Note the PE downclocks 2.4->2.0 GHz under the sustained fp8
DoubleRow power draw (P0), so DoubleRow nets ~1.6x, not 2x.

Measured: rel_err ~6.8e-3 (vs 2e-2 gate; fp16-PV baseline was 1.455e-2).
"""

import sys

import numpy as np

try:
    import concourse.bass as bass  # noqa: F401
except ImportError:  # pragma: no cover
    sys.path.insert(0, "/opt/trn_rl_repo")

import concourse.bacc as bacc
import concourse.mybir as mybir
import concourse.tile as tile
from concourse.masks import make_identity
from concourse import bass_utils

N_TOKENS = 8192
EMBED = 1024
NCORES = 8
M = N_TOKENS // NCORES  # rows per core (1024)
P = 128  # partitions
DC = EMBED // P  # contraction chunks (8)
DE = EMBED + 8  # x-ext width: embed + 8 ones-columns (u rows in G)
NB = 512  # key-block width
NNB = N_TOKENS // NB  # key blocks (16)
MB = M // P  # query row-blocks per core (8)
VC = NB // P  # value chunks per key block (4)
GR = DE  # Gram rows (1032: 1024 dims + 8 u-rows)
FP32 = mybir.dt.float32
BF16 = mybir.dt.bfloat16
FP16 = mybir.dt.float16
FP8 = mybir.dt.float8e4
EXP = mybir.ActivationFunctionType.Exp
IDN = mybir.ActivationFunctionType.Identity
ADD = mybir.AluOpType.add
SUB = mybir.AluOpType.subtract
DROW = mybir.MatmulPerfMode.DoubleRow
# logits scale: 1/sqrt(EMBED) softmax scale x 1/8 undoing the 8*Z prescale
SCALE = 1.0 / 256.0
MU = 1.088  # centering constant 1 + E[r(l)], l ~ N(0, 0.41^2)


def _build():
    nc = bacc.Bacc(
        "TRN2", target_bir_lowering=False, debug=False, num_devices=NCORES
    )
    xt_shard = nc.dram_tensor("xt_shard", [EMBED, M], FP16, kind="ExternalInput").ap()
    xs_shard = nc.dram_tensor("xs_shard", [M, DE], FP16, kind="ExternalInput").ap()
    xt8_full = nc.dram_tensor(
        "xt8_full", [EMBED, N_TOKENS], FP8, kind="ExternalInput"
    ).ap()
    xv8_full = nc.dram_tensor(
        "xv8_full", [N_TOKENS, EMBED], FP8, kind="ExternalInput"
    ).ap()
    z_d = nc.dram_tensor("z", [EMBED, EMBED], FP16, kind="ExternalInput").ap()
    out_d = nc.dram_tensor("out", [M, EMBED], BF16, kind="ExternalOutput").ap()

    z_r = z_d.rearrange("(a p) d -> a p d", p=P)  # [DC, P, EMBED]
    xt_r = xt_shard.rearrange("(a p) m -> a p m", p=P)  # [DC, P, M]
    xs_r = xs_shard.rearrange("(a p) d -> a p d", p=P)  # [DC, P, DE]
    xt8_r = xt8_full.rearrange("(a p) n -> a p n", p=P)  # [DC, P, N]
    xv_r = xv8_full.rearrange("(t p) d -> t p d", p=P)  # [64, P, EMBED]
    out_r = out_d.rearrange("(t p) d -> t p d", p=P)  # [MB, P, EMBED]

    with tile.TileContext(nc) as tc:
        with (
            tc.tile_pool(name="persist", bufs=1) as pers,
            tc.tile_pool(name="gdram", bufs=1, space="DRAM") as dpool,
            tc.tile_pool(name="gx", bufs=1) as gxp,
            tc.tile_pool(name="gst", bufs=4) as gst,
        ):
            ones8 = pers.tile([P, 2 * P], FP8)
            nc.vector.memset(ones8[:], 1.0)
            onesu = pers.tile([P, P], FP16)
            nc.vector.memset(onesu[:], MU / 8.0)
            mu_t = pers.tile([P, 1], FP32)
            nc.vector.memset(mu_t[:], MU)
            warm = pers.tile([P, 2], FP32)
            nc.vector.memset(warm[:], 0.0)
            # prime the ScalarE activation table while the head DMAs run
            nc.scalar.activation(out=warm[:, 1:2], in_=warm[:, 0:1],
                                 func=EXP, scale=SCALE)
            ident = pers.tile([P, P], FP32)
            make_identity(nc, ident[:])
            g8f = pers.tile([P, EMBED], FP32)  # u-rows staged fp32 (8 parts)
            # q~^T fp8 (scores stationary operand) and q~^T/256 fp16 (LV)
            qt8 = pers.tile([P, DC * M], FP8)
            qt16 = pers.tile([P, DC * M], FP16)
            # fp32 accumulator per query block: [p, mb*EMBED + dv]
            out_acc = pers.tile([P, MB * EMBED], FP32)
            # sum_k r contributions, replicated across partitions: [p, m]
            sums_acc = pers.tile([P, M], FP32)
            # AllReduce'd Gram: rows 0..1023 chunked, u-rows 1024..1031
            gsc = pers.tile([P, DC * EMBED], FP16)
            g8 = pers.tile([P, EMBED], FP16)
            # u along partitions (8 replicated cols per chunk), mu*u bcast
            ucol = pers.tile([P, DC * 8], FP16)
            ubc = pers.tile([P, EMBED], FP16)
            rsl = pers.tile([P, MB], FP32)
            g_in = dpool.tile([GR, EMBED], FP16)
            g_out = dpool.tile([GR, EMBED], FP16)
            xs_sb = gxp.tile([P, DC * DE], FP16)

            def g_group(pc, fh, psp, pstag):
                pw = P if pc < 8 else 8
                f0 = fh * NB
                ps = psp.tile([P, NB], FP32, tag=pstag, name=f"gps{pc}_{fh}")
                for a in range(DC):
                    nc.tensor.matmul(
                        ps[:pw, :],
                        lhsT=xs_sb[:, a * DE + pc * P : a * DE + pc * P + pw],
                        rhs=xs_sb[:, a * DE + f0 : a * DE + f0 + NB],
                        start=(a == 0),
                        stop=(a == DC - 1),
                    )
                gtile = gst.tile([P, NB], FP16, tag="gst", name=f"gt{pc}_{fh}")
                nc.vector.tensor_copy(out=gtile[:pw, :], in_=ps[:pw, :])
                nc.gpsimd.dma_start(
                    out=g_in[pc * P : pc * P + pw, f0 : f0 + NB],
                    in_=gtile[:pw, :],
                )

            # ---- Phase A: project q~^T = (8Z)^T @ x_own^T  (fp16), plus
            # the first Gram chunks; the rest fill pair 0's interleave
            # slots in phase B (pair 0 has no previous-pair R V work and
            # its score stream alone is ScalarE-bound).
            with (
                tc.tile_pool(name="proj", bufs=1) as proj,
                tc.tile_pool(name="proj_ps", bufs=4, space="PSUM") as proj_ps,
            ):
                z_sb = proj.tile([P, DC * EMBED], FP16)
                xt_sb = proj.tile([P, DC * M], FP16)
                # xs on the gpsimd queue: issues in parallel with sync's
                # z/xt stream, so the Gram head groups start PE early
                for a in range(DC):
                    nc.gpsimd.dma_start(
                        out=xs_sb[:, a * DE : (a + 1) * DE], in_=xs_r[a]
                    )
                for a in range(DC):
                    nc.sync.dma_start(
                        out=z_sb[:, a * EMBED : (a + 1) * EMBED], in_=z_r[a]
                    )
                    nc.sync.dma_start(
                        out=xt_sb[:, a * M : a * M + NB], in_=xt_r[a][:, 0:NB]
                    )
                for a in range(DC):
                    nc.sync.dma_start(
                        out=xt_sb[:, a * M + NB : (a + 1) * M],
                        in_=xt_r[a][:, NB:M],
                    )
                for pc in range(4):  # Gram head chunks lead while z/xt land
                    for fh in range(2):
                        g_group(pc, fh, proj_ps, "proj_ps")
                for j in range(M // NB):  # row half (j-outer: scores h=0
                    # needs every b of the j=0 half first)
                    for b in range(DC):  # output dim chunk
                        ps = proj_ps.tile([P, NB], FP32, tag="proj_ps")
                        for a in range(DC):  # contraction chunk
                            nc.tensor.matmul(
                                ps[:],
                                lhsT=z_sb[:, a * EMBED + b * P : a * EMBED + (b + 1) * P],
                                rhs=xt_sb[:, a * M + j * NB : a * M + (j + 1) * NB],
                                start=(a == 0),
                                stop=(a == DC - 1),
                            )
                        sl = slice(b * M + j * NB, b * M + (j + 1) * NB)
                        nc.vector.tensor_copy(out=qt8[:, sl], in_=ps[:])
                        nc.vector.tensor_scalar_mul(qt16[:, sl], ps[:], SCALE)

            g_fill = [(pc, fh) for pc in range(4, 9) for fh in range(2)]

            # ---- Phase B: streaming pass over the 16 key blocks.
            # Source-interleaved software pipeline: score groups of pair n
            # alternate with R V groups of pair n-1 (Gram groups at pair 0).
            with (
                tc.tile_pool(name="kv", bufs=3) as kvp,
                tc.tile_pool(name="rb", bufs=2) as rbp,
                tc.tile_pool(name="ex", bufs=6) as exp_,
                tc.tile_pool(name="ps_s", bufs=4, space="PSUM") as ps_sp,
                tc.tile_pool(name="ps_u", bufs=2, space="PSUM") as ps_up,
                tc.tile_pool(name="ps_o", bufs=2, space="PSUM") as ps_op,
                tc.tile_pool(name="fin", bufs=2) as fin,
                tc.tile_pool(name="outp", bufs=3) as outp,
            ):
                scol = fin.tile([P, MB], FP32)
                rtot = fin.tile([P, MB], FP32)
                ones2_v = ones8[:].rearrange("p (s q) -> p s q", s=2)
                qh_v = qt8[:].rearrange("p (b m) -> p b m", b=DC)  # [P, DC, M]

                def rv_group(rts, vts, np_, mb, h):
                    ps_o = ps_op.tile([P, NB], FP32, tag="ps_o", name=f"rv{mb}_{h}")
                    for blk in range(2):
                        r_v = rts[blk][:].rearrange("p (c m) -> p c m", c=VC)
                        v_v = vts[blk][:].rearrange("p (t e) -> p t e", t=VC)
                        for t2 in range(VC // 2):
                            nc.tensor.matmul(
                                ps_o[:],
                                lhsT=r_v[:, 2 * t2 : 2 * t2 + 2, mb * P : (mb + 1) * P],
                                rhs=v_v[:, 2 * t2 : 2 * t2 + 2, h * NB : (h + 1) * NB],
                                start=(blk == 0 and t2 == 0),
                                stop=(blk == 1 and t2 == VC // 2 - 1),
                                perf_mode=DROW,
                            )
                    dst = out_acc[:, mb * EMBED + h * NB : mb * EMBED + (h + 1) * NB]
                    if np_ == 0:
                        nc.vector.tensor_copy(out=dst, in_=ps_o[:])
                    else:
                        nc.vector.tensor_tensor(out=dst, in0=dst, in1=ps_o[:], op=ADD)

                def sums_group(rts, np_, h):
                    ps_sum = ps_up.tile([P, NB], FP32, tag="ps_sum", name=f"su{h}")
                    for blk in range(2):
                        r_v = rts[blk][:].rearrange("p (c m) -> p c m", c=VC)
                        for cc in range(VC // 2):
                            nc.tensor.matmul(
                                ps_sum[:],
                                lhsT=ones2_v,
                                rhs=r_v[:, 2 * cc : 2 * cc + 2, h * NB : (h + 1) * NB],
                                start=(blk == 0 and cc == 0),
                                stop=(blk == 1 and cc == VC // 2 - 1),
                                perf_mode=DROW,
                            )
                    dsts = sums_acc[:, h * NB : (h + 1) * NB]
                    if np_ == 0:
                        nc.vector.tensor_copy(out=dsts, in_=ps_sum[:])
                    else:
                        nc.vector.tensor_tensor(
                            out=dsts, in0=dsts, in1=ps_sum[:], op=ADD
                        )

                prev = None  # (rts, vts, np_) of previous pair
                for np_ in range(NNB // 2):
                    rts, vts = [], []
                    kt_vs = []
                    for blk in range(2):
                        nb = 2 * np_ + blk
                        vtile = kvp.tile([P, VC * EMBED], FP8, tag=f"vt{blk}")
                        for c in range(VC):
                            nc.sync.dma_start(
                                out=vtile[:, c * EMBED : (c + 1) * EMBED],
                                in_=xv_r[nb * VC + c],
                            )
                        ktile = kvp.tile([P, DC * NB], FP8, tag=f"kt{blk}")
                        for b in range(DC):
                            nc.sync.dma_start(
                                out=ktile[:, b * NB : (b + 1) * NB],
                                in_=xt8_r[b, :, nb * NB : (nb + 1) * NB],
                            )
                        kt_vs.append(ktile[:].rearrange("p (b n) -> p b n", b=DC))
                        rtile = rbp.tile([P, VC * M], FP8, tag=f"rt{blk}")
                        rts.append(rtile)
                        vts.append(vtile)

                    # interleave: 16 score groups with fill work (prev RV,
                    # or Gram groups at pair 0)
                    if prev is not None:
                        prts, pvts, pnp = prev
                        fill = [
                            (lambda mb=mb, h=h: rv_group(prts, pvts, pnp, mb, h))
                            for mb in range(MB) for h in range(EMBED // NB)
                        ]
                    else:
                        fill = [
                            (lambda pc=pc, fh=fh: g_group(pc, fh, ps_op, "ps_o"))
                            for (pc, fh) in g_fill
                        ]
                    fi = 0
                    for blk in range(2):
                        for h in range(M // NB):  # query column half
                            for c in range(VC):  # key chunk within block
                                ps_s = ps_sp.tile([P, NB], FP32, tag="ps_s")
                                for bb in range(DC // 2):
                                    nc.tensor.matmul(
                                        ps_s[:],
                                        lhsT=kt_vs[blk][
                                            :, 2 * bb : 2 * bb + 2, c * P : (c + 1) * P
                                        ],
                                        rhs=qh_v[
                                            :, 2 * bb : 2 * bb + 2,
                                            h * NB : (h + 1) * NB,
                                        ],
                                        start=(bb == 0),
                                        stop=(bb == DC // 2 - 1),
                                        perf_mode=DROW,
                                    )
                                csl = slice(c * M + h * NB, c * M + (h + 1) * NB)
                                pe = exp_.tile([P, NB], FP16, tag="pe")
                                nc.scalar.activation(
                                    out=pe[:], in_=ps_s[:], func=EXP, scale=SCALE
                                )
                                tl = exp_.tile([P, NB], FP16, tag="tl")
                                nc.scalar.activation(
                                    out=tl[:], in_=ps_s[:], func=IDN,
                                    scale=SCALE, bias=mu_t[:],
                                )
                                nc.vector.tensor_tensor(
                                    out=rts[blk][:, csl], in0=pe[:], in1=tl[:], op=SUB
                                )
                                if fi < len(fill):
                                    fill[fi]()
                                    fi += 1
                    while fi < len(fill):
                        fill[fi]()
                        fi += 1
                    if prev is not None:
                        for h in range(M // NB):
                            sums_group(prts, pnp, h)

                    if np_ == 0:
                        # all Gram partials written: fire the AllReduce and
                        # pull the result back on the gpsimd queue (never
                        # blocks the sync-queue K/V stream)
                        nc.gpsimd.collective_compute(
                            "AllReduce",
                            mybir.AluOpType.add,
                            replica_groups=[list(range(NCORES))],
                            ins=[g_in[:].opt()],
                            outs=[g_out[:].opt()],
                        )
                        for a in range(DC):
                            nc.gpsimd.dma_start(
                                out=gsc[:, a * EMBED : (a + 1) * EMBED],
                                in_=g_out[a * P : (a + 1) * P, :],
                            )
                        nc.gpsimd.dma_start(out=g8[:8, :], in_=g_out[EMBED:GR, :])

                    if np_ == 5:
                        # AllReduce long done: exact linear terms while the
                        # stream continues.  u-cols from transposed u-rows.
                        nc.vector.tensor_copy(out=g8f[:8, :], in_=g8[:8, :])
                        for a in range(DC):
                            ps_t = ps_up.tile([P, NB], FP32, tag="ps_sum")
                            nc.tensor.transpose(
                                out=ps_t[:, 0:8],
                                in_=g8f[:8, a * P : (a + 1) * P],
                                identity=ident[:8, :8],
                            )
                            nc.vector.tensor_copy(
                                out=ucol[:, a * 8 : (a + 1) * 8], in_=ps_t[:, 0:8]
                            )
                        for mb in range(MB):
                            for fh in range(EMBED // NB):
                                ps_l = ps_op.tile([P, NB], FP32, tag="ps_o")
                                for a in range(DC):
                                    nc.tensor.matmul(
                                        ps_l[:],
                                        lhsT=qt16[:, a * M + mb * P : a * M + (mb + 1) * P],
                                        rhs=gsc[:, a * EMBED + fh * NB : a * EMBED + (fh + 1) * NB],
                                        start=(a == 0),
                                        stop=(a == DC - 1),
                                    )
                                dstl = out_acc[
                                    :, mb * EMBED + fh * NB : mb * EMBED + (fh + 1) * NB
                                ]
                                nc.vector.tensor_tensor(
                                    out=dstl, in0=dstl, in1=ps_l[:], op=ADD
                                )
                            ps_r = ps_up.tile([P, NB], FP32, tag="ps_sum")
                            for a in range(DC):
                                nc.tensor.matmul(
                                    ps_r[:, :8],
                                    lhsT=qt16[:, a * M + mb * P : a * M + (mb + 1) * P],
                                    rhs=ucol[:, a * 8 : (a + 1) * 8],
                                    start=(a == 0),
                                    stop=(a == DC - 1),
                                )
                            nc.vector.tensor_copy(
                                out=rsl[:, mb : mb + 1], in_=ps_r[:, 0:1]
                            )
                        for fh in range(EMBED // NB):
                            ps_b = ps_op.tile([P, NB], FP32, tag="ps_o")
                            nc.tensor.matmul(
                                ps_b[:],
                                lhsT=onesu[:8, :],
                                rhs=g8[:8, fh * NB : (fh + 1) * NB],
                                start=True,
                                stop=True,
                            )
                            nc.vector.tensor_copy(
                                out=ubc[:, fh * NB : (fh + 1) * NB], in_=ps_b[:]
                            )
                        for mb in range(MB):
                            dstu = out_acc[:, mb * EMBED : (mb + 1) * EMBED]
                            nc.vector.tensor_tensor(
                                out=dstu, in0=dstu, in1=ubc[:], op=ADD
                            )

                    prev = (rts, vts, np_)

                # tail: last pair's sums, denominators, then RV + divide
                prts, pvts, pnp = prev
                for h in range(M // NB):
                    sums_group(prts, pnp, h)
                for mb in range(MB):
                    ps_f = ps_up.tile([P, NB], FP32, tag="ps_sum")
                    nc.tensor.transpose(
                        out=ps_f[:, 0:P],
                        in_=sums_acc[:, mb * P : (mb + 1) * P],
                        identity=ident[:],
                    )
                    nc.vector.tensor_copy(
                        out=scol[:, mb : mb + 1], in_=ps_f[:, 0:1]
                    )
                nc.vector.tensor_tensor(out=scol[:], in0=scol[:], in1=rsl[:], op=ADD)
                nc.vector.tensor_scalar_add(scol[:], scol[:], MU * N_TOKENS)
                nc.vector.reciprocal(out=rtot[:], in_=scol[:])
                for mb in range(MB):
                    for h in range(EMBED // NB):
                        rv_group(prts, pvts, pnp, mb, h)
                    outf = outp.tile([P, EMBED], BF16, tag="outf")
                    nc.vector.tensor_scalar_mul(
                        outf[:],
                        out_acc[:, mb * EMBED : (mb + 1) * EMBED],
                        rtot[:, mb : mb + 1],
                    )
                    nc.sync.dma_start(out=out_r[mb], in_=outf[:])

    nc.compile()
    return nc


_NC = None


def _get_nc():
    global _NC
    if _NC is None:
        _NC = _build()
    return _NC


def _run(x, rotation_params, entangle_params, **spmd_kwargs):
    x = np.ascontiguousarray(np.asarray(x, dtype=np.float32))
    wq = np.asarray(rotation_params, dtype=np.float32).reshape(EMBED, EMBED)
    wk = np.asarray(entangle_params, dtype=np.float32).reshape(EMBED, EMBED)
    import ml_dtypes

    # offline weight folding: Z = 8 * Wq Wk^T (the 8x keeps the fp8 q~ in
    # e4m3's normal range; undone in the exp/linear scales)
    z8 = (8.0 * (wq @ wk.T)).astype(np.float16)
    xt = np.ascontiguousarray(x.T)
    xt16 = xt.astype(np.float16)
    xt8 = xt.astype(ml_dtypes.float8_e4m3)
    xv8 = x.astype(ml_dtypes.float8_e4m3)
    x16 = x.astype(np.float16)
    ones8c = np.ones((M, 8), np.float16)
    in_maps = [
        {
            "xt_shard": np.ascontiguousarray(xt16[:, i * M : (i + 1) * M]),
            "xs_shard": np.ascontiguousarray(
                np.concatenate([x16[i * M : (i + 1) * M], ones8c], axis=1)
            ),
            "xt8_full": xt8,
            "xv8_full": xv8,
            "z": z8,
        }
        for i in range(NCORES)
    ]
    res = bass_utils.run_bass_kernel_spmd(
        _get_nc(), in_maps, core_ids=list(range(NCORES)), **spmd_kwargs
    )
    out = np.concatenate(
        [res.results[i]["out"].astype(np.float32) for i in range(NCORES)], axis=0
    )
    return out, res


def kernel(x, rotation_params, entangle_params):
    out, _ = _run(x, rotation_params, entangle_params)
    return out


# revision 20
# speedup vs baseline: 1.0979x; 1.0079x over previous
"""Trainium2 Bass kernel for ClassicalSelfAttention.

  out = softmax((x @ Wq) @ (x @ Wk)^T / sqrt(D)) @ x      x: [8192, 1024] f32

Algebraic restructuring 1 (weight folding): scores = x (Wq Wk^T) x^T; the
weight matrices fold offline into Z = Wq Wk^T.  Each core projects only
its own row-shard (q~ = x_own @ 8Z) and computes its scores row-block
directly against x^T streamed in fp8.  No K projection, no K^T AllGather.

Algebraic restructuring 2 (LINEAR SPLIT): the logits are small
(l ~ N(0, 0.41^2)), so exp(l) = 1 + l + r(l) with the curvature
remainder r = e^l - 1 - l tiny (rms ~0.12 vs ~1.18 for e^l).  The
attention numerator splits exactly:

   P V = mu * colsum(V)  +  q~ (X^T X) / 256  +  R V

The linear term collapses through the D x D Gram matrix G = X^T X (each
core computes its shard's partial; one fp16 AllReduce, overlapped with
the main stream), and only the centered remainder R needs the N^2 D
matmul -- it quantizes to fp8 ~10x better than P, so BOTH big matmuls
(scores and R V) run fp8 DoubleRow at 2x-per-instruction PE rate.  fp8
score errors only perturb the second-order term (p-1)*dl, since the
linear term is exact through G -- the scheme self-corrects the linear
part of all score quantization noise, which is why the scores need no
hi/lo residual pass.  Denominators: s = mu*N + q~ u/256 + sum_k r, with
u = X^T 1 riding along in the Gram AllReduce as 8 extra lhsT ones-rows.

Engine schedule: each score tile costs ScalarE TWO passes (exp and the
linear map t = l + mu feeding R = p - t), which exceeds the PE's fp8
issue rate -- so score groups of pair n are source-interleaved with the
R V groups of pair n-1, and pair 0's slack is filled with the Gram
matmuls.  All AllReduce bounce DMAs ride the gpsimd queue so the
collective-gated read-back never blocks the K/V stream on the sync
queues.  Note the PE downclocks 2.4->2.0 GHz under the sustained fp8
DoubleRow power draw (P0), so DoubleRow nets ~1.6x, not 2x.

Measured: rel_err ~6.8e-3 (vs 2e-2 gate; fp16-PV baseline was 1.455e-2).
"""

import sys

import numpy as np

try:
    import concourse.bass as bass  # noqa: F401
except ImportError:  # pragma: no cover
    sys.path.insert(0, "/opt/trn_rl_repo")

import concourse.bacc as bacc
import concourse.mybir as mybir
import concourse.tile as tile
from concourse.masks import make_identity
from concourse import bass_utils

N_TOKENS = 8192
EMBED = 1024
NCORES = 8
M = N_TOKENS // NCORES  # rows per core (1024)
P = 128  # partitions
DC = EMBED // P  # contraction chunks (8)
DE = EMBED  # xs shard width (u comes from a DVE row-sum instead)
NB = 512  # key-block width
NNB = N_TOKENS // NB  # key blocks (16)
MB = M // P  # query row-blocks per core (8)
VC = NB // P  # value chunks per key block (4)
GR = EMBED + 1  # Gram rows: 1024 dims + 1 u-row
FP32 = mybir.dt.float32
BF16 = mybir.dt.bfloat16
FP16 = mybir.dt.float16
FP8 = mybir.dt.float8e4
EXP = mybir.ActivationFunctionType.Exp
IDN = mybir.ActivationFunctionType.Identity
ADD = mybir.AluOpType.add
SUB = mybir.AluOpType.subtract
DROW = mybir.MatmulPerfMode.DoubleRow
# logits scale: 1/sqrt(EMBED) softmax scale x 1/8 undoing the 8*Z prescale
SCALE = 1.0 / 256.0
MU = 1.088  # centering constant 1 + E[r(l)], l ~ N(0, 0.41^2)


def _build():
    nc = bacc.Bacc(
        "TRN2", target_bir_lowering=False, debug=False, num_devices=NCORES
    )
    xt_shard = nc.dram_tensor("xt_shard", [EMBED, M], FP16, kind="ExternalInput").ap()
    xs_shard = nc.dram_tensor("xs_shard", [M, DE], FP16, kind="ExternalInput").ap()
    xt8_full = nc.dram_tensor(
        "xt8_full", [EMBED, N_TOKENS], FP8, kind="ExternalInput"
    ).ap()
    xv8_full = nc.dram_tensor(
        "xv8_full", [N_TOKENS, EMBED], FP8, kind="ExternalInput"
    ).ap()
    z_d = nc.dram_tensor("z", [EMBED, EMBED], FP16, kind="ExternalInput").ap()
    out_d = nc.dram_tensor("out", [M, EMBED], BF16, kind="ExternalOutput").ap()

    z_r = z_d.rearrange("(a p) d -> a p d", p=P)  # [DC, P, EMBED]
    xt_r = xt_shard.rearrange("(a p) m -> a p m", p=P)  # [DC, P, M]
    xs_r = xs_shard.rearrange("(a p) d -> a p d", p=P)  # [DC, P, DE]
    xt8_r = xt8_full.rearrange("(a p) n -> a p n", p=P)  # [DC, P, N]
    xv_r = xv8_full.rearrange("(t p) d -> t p d", p=P)  # [64, P, EMBED]
    out_r = out_d.rearrange("(t p) d -> t p d", p=P)  # [MB, P, EMBED]

    with tile.TileContext(nc) as tc:
        with (
            tc.tile_pool(name="persist", bufs=1) as pers,
            tc.tile_pool(name="gdram", bufs=1, space="DRAM") as dpool,
            tc.tile_pool(name="gx", bufs=1) as gxp,
            tc.tile_pool(name="gst", bufs=4) as gst,
        ):
            ones8 = pers.tile([P, 2 * P], FP8)
            nc.vector.memset(ones8[:], 1.0)
            onesu = pers.tile([P, P], FP16)
            nc.vector.memset(onesu[:], MU)
            mu_t = pers.tile([P, 1], FP32)
            nc.vector.memset(mu_t[:], MU)
            warm = pers.tile([P, 2], FP32)
            nc.vector.memset(warm[:], 0.0)
            # prime the ScalarE activation table while the head DMAs run
            nc.scalar.activation(out=warm[:, 1:2], in_=warm[:, 0:1],
                                 func=EXP, scale=SCALE)
            ident = pers.tile([P, P], FP32)
            make_identity(nc, ident[:])
            g8f = pers.tile([P, EMBED], FP32)  # u-rows staged fp32 (8 parts)
            # q~^T fp8 (scores stationary operand) and q~^T/256 fp16 (LV)
            qt8 = pers.tile([P, DC * M], FP8)
            qt16 = pers.tile([P, DC * M], FP16)
            # fp32 accumulator per query block: [p, mb*EMBED + dv]
            out_acc = pers.tile([P, MB * EMBED], FP32)
            # sum_k r contributions, replicated across partitions: [p, m]
            sums_acc = pers.tile([P, M], FP32)
            # AllReduce'd Gram: rows 0..1023 chunked, u-rows 1024..1031
            gsc = pers.tile([P, DC * EMBED], FP16)
            g8 = pers.tile([P, EMBED], FP16)
            # u along partitions (8 replicated cols per chunk), mu*u bcast
            ucol = pers.tile([P, DC], FP16)
            ubc = pers.tile([P, EMBED], FP16)
            rsl = pers.tile([P, MB], FP32)
            g_in = dpool.tile([GR, EMBED], FP16)
            g_out = dpool.tile([GR, EMBED], FP16)
            xs_sb = gxp.tile([P, DC * DE], FP16)

            def g_group(pc, fh, psp, pstag):
                pw = P
                f0 = fh * NB
                ps = psp.tile([P, NB], FP32, tag=pstag, name=f"gps{pc}_{fh}")
                for a in range(DC):
                    nc.tensor.matmul(
                        ps[:pw, :],
                        lhsT=xs_sb[:, a * DE + pc * P : a * DE + pc * P + pw],
                        rhs=xs_sb[:, a * DE + f0 : a * DE + f0 + NB],
                        start=(a == 0),
                        stop=(a == DC - 1),
                    )
                gtile = gst.tile([P, NB], FP16, tag="gst", name=f"gt{pc}_{fh}")
                nc.vector.tensor_copy(out=gtile[:pw, :], in_=ps[:pw, :])
                nc.gpsimd.dma_start(
                    out=g_in[pc * P : pc * P + pw, f0 : f0 + NB],
                    in_=gtile[:pw, :],
                )

            # ---- Phase A: project q~^T = (8Z)^T @ x_own^T  (fp16), plus
            # the first Gram chunks; the rest fill pair 0's interleave
            # slots in phase B (pair 0 has no previous-pair R V work and
            # its score stream alone is ScalarE-bound).
            with (
                tc.tile_pool(name="proj", bufs=1) as proj,
                tc.tile_pool(name="proj_ps", bufs=4, space="PSUM") as proj_ps,
            ):
                z_sb = proj.tile([P, DC * EMBED], FP16)
                xt_sb = proj.tile([P, DC * M], FP16)
                # xs on the gpsimd queue: issues in parallel with sync's
                # z/xt stream, so the Gram head groups start PE early
                for a in range(DC):
                    nc.gpsimd.dma_start(
                        out=xs_sb[:, a * DE : (a + 1) * DE], in_=xs_r[a]
                    )
                for a in range(DC):
                    nc.sync.dma_start(
                        out=z_sb[:, a * EMBED : (a + 1) * EMBED], in_=z_r[a]
                    )
                    nc.sync.dma_start(
                        out=xt_sb[:, a * M : a * M + NB], in_=xt_r[a][:, 0:NB]
                    )
                for a in range(DC):
                    nc.sync.dma_start(
                        out=xt_sb[:, a * M + NB : (a + 1) * M],
                        in_=xt_r[a][:, NB:M],
                    )
                # u-partial = row-sums of the shard, via DVE (free-axis
                # reduce over xt^T), rides the Gram AllReduce as row 1024
                ured32 = proj.tile([P, DC], FP32)
                ured16 = proj.tile([P, DC], FP16)
                for a in range(DC):
                    nc.vector.tensor_reduce(
                        out=ured32[:, a : a + 1],
                        in_=xt_sb[:, a * M : (a + 1) * M],
                        axis=mybir.AxisListType.XYZW,
                        op=ADD,
                    )
                nc.vector.tensor_copy(out=ured16[:], in_=ured32[:])
                for a in range(DC):
                    nc.gpsimd.dma_start(
                        out=g_in[EMBED : EMBED + 1, a * P : (a + 1) * P],
                        in_=ured16[:, a : a + 1],
                    )
                for pc in range(4):  # Gram head chunks lead while z/xt land
                    for fh in range(2):
                        g_group(pc, fh, proj_ps, "proj_ps")
                for j in range(M // NB):  # row half (j-outer: scores h=0
                    # needs every b of the j=0 half first)
                    for b in range(DC):  # output dim chunk
                        ps = proj_ps.tile([P, NB], FP32, tag="proj_ps")
                        for a in range(DC):  # contraction chunk
                            nc.tensor.matmul(
                                ps[:],
                                lhsT=z_sb[:, a * EMBED + b * P : a * EMBED + (b + 1) * P],
                                rhs=xt_sb[:, a * M + j * NB : a * M + (j + 1) * NB],
                                start=(a == 0),
                                stop=(a == DC - 1),
                            )
                        sl = slice(b * M + j * NB, b * M + (j + 1) * NB)
                        nc.vector.tensor_copy(out=qt8[:, sl], in_=ps[:])
                        nc.vector.tensor_scalar_mul(qt16[:, sl], ps[:], SCALE)

            g_fill = [(pc, fh) for pc in range(4, 8) for fh in range(2)]

            # ---- Phase B: streaming pass over the 16 key blocks.
            # Source-interleaved software pipeline: score groups of pair n
            # alternate with R V groups of pair n-1 (Gram groups at pair 0).
            with (
                tc.tile_pool(name="kv", bufs=3) as kvp,
                tc.tile_pool(name="rb", bufs=2) as rbp,
                tc.tile_pool(name="ex", bufs=6) as exp_,
                tc.tile_pool(name="ps_s", bufs=4, space="PSUM") as ps_sp,
                tc.tile_pool(name="ps_u", bufs=2, space="PSUM") as ps_up,
                tc.tile_pool(name="ps_o", bufs=2, space="PSUM") as ps_op,
                tc.tile_pool(name="fin", bufs=2) as fin,
                tc.tile_pool(name="outp", bufs=3) as outp,
            ):
                scol = fin.tile([P, MB], FP32)
                rtot = fin.tile([P, MB], FP32)
                ones2_v = ones8[:].rearrange("p (s q) -> p s q", s=2)
                qh_v = qt8[:].rearrange("p (b m) -> p b m", b=DC)  # [P, DC, M]

                def rv_group(rts, vts, np_, mb, h):
                    ps_o = ps_op.tile([P, NB], FP32, tag="ps_o", name=f"rv{mb}_{h}")
                    for blk in range(2):
                        r_v = rts[blk][:].rearrange("p (c m) -> p c m", c=VC)
                        v_v = vts[blk][:].rearrange("p (t e) -> p t e", t=VC)
                        for t2 in range(VC // 2):
                            nc.tensor.matmul(
                                ps_o[:],
                                lhsT=r_v[:, 2 * t2 : 2 * t2 + 2, mb * P : (mb + 1) * P],
                                rhs=v_v[:, 2 * t2 : 2 * t2 + 2, h * NB : (h + 1) * NB],
                                start=(blk == 0 and t2 == 0),
                                stop=(blk == 1 and t2 == VC // 2 - 1),
                                perf_mode=DROW,
                            )
                    dst = out_acc[:, mb * EMBED + h * NB : mb * EMBED + (h + 1) * NB]
                    if np_ == 0:
                        nc.vector.tensor_copy(out=dst, in_=ps_o[:])
                    else:
                        nc.vector.tensor_tensor(out=dst, in0=dst, in1=ps_o[:], op=ADD)

                def sums_group(rts, np_, h):
                    ps_sum = ps_up.tile([P, NB], FP32, tag="ps_sum", name=f"su{h}")
                    for blk in range(2):
                        r_v = rts[blk][:].rearrange("p (c m) -> p c m", c=VC)
                        for cc in range(VC // 2):
                            nc.tensor.matmul(
                                ps_sum[:],
                                lhsT=ones2_v,
                                rhs=r_v[:, 2 * cc : 2 * cc + 2, h * NB : (h + 1) * NB],
                                start=(blk == 0 and cc == 0),
                                stop=(blk == 1 and cc == VC // 2 - 1),
                                perf_mode=DROW,
                            )
                    dsts = sums_acc[:, h * NB : (h + 1) * NB]
                    if np_ == 0:
                        nc.vector.tensor_copy(out=dsts, in_=ps_sum[:])
                    else:
                        nc.vector.tensor_tensor(
                            out=dsts, in0=dsts, in1=ps_sum[:], op=ADD
                        )

                prev = None  # (rts, vts, np_) of previous pair
                for np_ in range(NNB // 2):
                    rts, vts = [], []
                    kt_vs = []
                    for blk in range(2):
                        nb = 2 * np_ + blk
                        vtile = kvp.tile([P, VC * EMBED], FP8, tag=f"vt{blk}")
                        for c in range(VC):
                            nc.sync.dma_start(
                                out=vtile[:, c * EMBED : (c + 1) * EMBED],
                                in_=xv_r[nb * VC + c],
                            )
                        ktile = kvp.tile([P, DC * NB], FP8, tag=f"kt{blk}")
                        for b in range(DC):
                            nc.sync.dma_start(
                                out=ktile[:, b * NB : (b + 1) * NB],
                                in_=xt8_r[b, :, nb * NB : (nb + 1) * NB],
                            )
                        kt_vs.append(ktile[:].rearrange("p (b n) -> p b n", b=DC))
                        rtile = rbp.tile([P, VC * M], FP8, tag=f"rt{blk}")
                        rts.append(rtile)
                        vts.append(vtile)

                    # interleave: 16 score groups with fill work (prev RV,
                    # or Gram groups at pair 0)
                    if prev is not None:
                        prts, pvts, pnp = prev
                        fill = [
                            (lambda mb=mb, h=h: rv_group(prts, pvts, pnp, mb, h))
                            for mb in range(MB) for h in range(EMBED // NB)
                        ]
                    else:
                        fill = [
                            (lambda pc=pc, fh=fh: g_group(pc, fh, ps_op, "ps_o"))
                            for (pc, fh) in g_fill
                        ]
                    fi = 0
                    for blk in range(2):
                        for h in range(M // NB):  # query column half
                            for c in range(VC):  # key chunk within block
                                ps_s = ps_sp.tile([P, NB], FP32, tag="ps_s")
                                for bb in range(DC // 2):
                                    nc.tensor.matmul(
                                        ps_s[:],
                                        lhsT=kt_vs[blk][
                                            :, 2 * bb : 2 * bb + 2, c * P : (c + 1) * P
                                        ],
                                        rhs=qh_v[
                                            :, 2 * bb : 2 * bb + 2,
                                            h * NB : (h + 1) * NB,
                                        ],
                                        start=(bb == 0),
                                        stop=(bb == DC // 2 - 1),
                                        perf_mode=DROW,
                                    )
                                csl = slice(c * M + h * NB, c * M + (h + 1) * NB)
                                pe = exp_.tile([P, NB], FP16, tag="pe")
                                nc.scalar.activation(
                                    out=pe[:], in_=ps_s[:], func=EXP, scale=SCALE
                                )
                                tl = exp_.tile([P, NB], FP16, tag="tl")
                                nc.scalar.activation(
                                    out=tl[:], in_=ps_s[:], func=IDN,
                                    scale=SCALE, bias=mu_t[:],
                                )
                                nc.vector.tensor_tensor(
                                    out=rts[blk][:, csl], in0=pe[:], in1=tl[:], op=SUB
                                )
                                if fi < len(fill):
                                    fill[fi]()
                                    fi += 1
                    while fi < len(fill):
                        fill[fi]()
                        fi += 1
                    if prev is not None:
                        for h in range(M // NB):
                            sums_group(prts, pnp, h)

                    if np_ == 0:
                        # all Gram partials written: fire the AllReduce and
                        # pull the result back on the gpsimd queue (never
                        # blocks the sync-queue K/V stream)
                        nc.gpsimd.collective_compute(
                            "AllReduce",
                            mybir.AluOpType.add,
                            replica_groups=[list(range(NCORES))],
                            ins=[g_in[:].opt()],
                            outs=[g_out[:].opt()],
                        )
                        for a in range(DC):
                            nc.gpsimd.dma_start(
                                out=gsc[:, a * EMBED : (a + 1) * EMBED],
                                in_=g_out[a * P : (a + 1) * P, :],
                            )
                        nc.gpsimd.dma_start(out=g8[:1, :], in_=g_out[EMBED:GR, :])

                    if np_ == 5:
                        # AllReduce long done: exact linear terms while the
                        # stream continues.  u-cols from transposed u-rows.
                        nc.vector.tensor_copy(out=g8f[:1, :], in_=g8[:1, :])
                        for a in range(DC):
                            ps_t = ps_up.tile([P, NB], FP32, tag="ps_sum")
                            nc.tensor.transpose(
                                out=ps_t[:, 0:1],
                                in_=g8f[:1, a * P : (a + 1) * P],
                                identity=ident[:1, :1],
                            )
                            nc.vector.tensor_copy(
                                out=ucol[:, a : a + 1], in_=ps_t[:, 0:1]
                            )
                        for mb in range(MB):
                            for fh in range(EMBED // NB):
                                ps_l = ps_op.tile([P, NB], FP32, tag="ps_o")
                                for a in range(DC):
                                    nc.tensor.matmul(
                                        ps_l[:],
                                        lhsT=qt16[:, a * M + mb * P : a * M + (mb + 1) * P],
                                        rhs=gsc[:, a * EMBED + fh * NB : a * EMBED + (fh + 1) * NB],
                                        start=(a == 0),
                                        stop=(a == DC - 1),
                                    )
                                dstl = out_acc[
                                    :, mb * EMBED + fh * NB : mb * EMBED + (fh + 1) * NB
                                ]
                                nc.vector.tensor_tensor(
                                    out=dstl, in0=dstl, in1=ps_l[:], op=ADD
                                )
                            ps_r = ps_up.tile([P, NB], FP32, tag="ps_sum")
                            for a in range(DC):
                                nc.tensor.matmul(
                                    ps_r[:, :1],
                                    lhsT=qt16[:, a * M + mb * P : a * M + (mb + 1) * P],
                                    rhs=ucol[:, a : a + 1],
                                    start=(a == 0),
                                    stop=(a == DC - 1),
                                )
                            nc.vector.tensor_copy(
                                out=rsl[:, mb : mb + 1], in_=ps_r[:, 0:1]
                            )
                        for fh in range(EMBED // NB):
                            ps_b = ps_op.tile([P, NB], FP32, tag="ps_o")
                            nc.tensor.matmul(
                                ps_b[:],
                                lhsT=onesu[:1, :],
                                rhs=g8[:1, fh * NB : (fh + 1) * NB],
                                start=True,
                                stop=True,
                            )
                            nc.vector.tensor_copy(
                                out=ubc[:, fh * NB : (fh + 1) * NB], in_=ps_b[:]
                            )
                        for mb in range(MB):
                            dstu = out_acc[:, mb * EMBED : (mb + 1) * EMBED]
                            nc.vector.tensor_tensor(
                                out=dstu, in0=dstu, in1=ubc[:], op=ADD
                            )

                    prev = (rts, vts, np_)

                # tail: last pair's sums, denominators, then RV + divide
                prts, pvts, pnp = prev
                for h in range(M // NB):
                    sums_group(prts, pnp, h)
                for mb in range(MB):
                    ps_f = ps_up.tile([P, NB], FP32, tag="ps_sum")
                    nc.tensor.transpose(
                        out=ps_f[:, 0:P],
                        in_=sums_acc[:, mb * P : (mb + 1) * P],
                        identity=ident[:],
                    )
                    nc.vector.tensor_copy(
                        out=scol[:, mb : mb + 1], in_=ps_f[:, 0:1]
                    )
                nc.vector.tensor_tensor(out=scol[:], in0=scol[:], in1=rsl[:], op=ADD)
                nc.vector.tensor_scalar_add(scol[:], scol[:], MU * N_TOKENS)
                nc.vector.reciprocal(out=rtot[:], in_=scol[:])
                for mb in range(MB):
                    for h in range(EMBED // NB):
                        rv_group(prts, pvts, pnp, mb, h)
                    outf = outp.tile([P, EMBED], BF16, tag="outf")
                    nc.vector.tensor_scalar_mul(
                        outf[:],
                        out_acc[:, mb * EMBED : (mb + 1) * EMBED],
                        rtot[:, mb : mb + 1],
                    )
                    nc.sync.dma_start(out=out_r[mb], in_=outf[:])

    nc.compile()
    return nc


_NC = None


def _get_nc():
    global _NC
    if _NC is None:
        _NC = _build()
    return _NC


def _run(x, rotation_params, entangle_params, **spmd_kwargs):
    x = np.ascontiguousarray(np.asarray(x, dtype=np.float32))
    wq = np.asarray(rotation_params, dtype=np.float32).reshape(EMBED, EMBED)
    wk = np.asarray(entangle_params, dtype=np.float32).reshape(EMBED, EMBED)
    import ml_dtypes

    # offline weight folding: Z = 8 * Wq Wk^T (the 8x keeps the fp8 q~ in
    # e4m3's normal range; undone in the exp/linear scales)
    z8 = (8.0 * (wq @ wk.T)).astype(np.float16)
    xt = np.ascontiguousarray(x.T)
    xt16 = xt.astype(np.float16)
    xt8 = xt.astype(ml_dtypes.float8_e4m3)
    xv8 = x.astype(ml_dtypes.float8_e4m3)
    x16 = x.astype(np.float16)
    in_maps = [
        {
            "xt_shard": np.ascontiguousarray(xt16[:, i * M : (i + 1) * M]),
            "xs_shard": np.ascontiguousarray(x16[i * M : (i + 1) * M]),
            "xt8_full": xt8,
            "xv8_full": xv8,
            "z": z8,
        }
        for i in range(NCORES)
    ]
    res = bass_utils.run_bass_kernel_spmd(
        _get_nc(), in_maps, core_ids=list(range(NCORES)), **spmd_kwargs
    )
    out = np.concatenate(
        [res.results[i]["out"].astype(np.float32) for i in range(NCORES)], axis=0
    )
    return out, res


def kernel(x, rotation_params, entangle_params):
    out, _ = _run(x, rotation_params, entangle_params)
    return out
